# revision 57
# baseline (speedup 1.0000x reference)
"""Trainium2 Bass kernel for nn_Caps1D (capsule routing, 3 iterations).

Sharding: pure data-parallel over batch B=1024 across 8 cores (128/core).
W is replicated. Output [1024, 2] gathered from per-core [128, 2].

Algorithm (per core):
  u_ji[b,r,o] = sum_i u[b,r,i] W[k,r,i,o]            (never materialized)
  Routing logit is linear in the squash history:
    L_t[b,r] = sum_o u_ji[b,r,o] * M_t[b,o],  M_t = sum_{t'<=t} v_n,t'
  so no logit accumulator field is kept; each L evaluation is
    PM = MpadT_k^T @ wotN-chunks  (PE outer, natural [b,(i,r)] PSUM)
    L  = sum_i u_im (.) PM        (DVE mul + 2 adds, i-planar)
    c~ = exp(L), Z via accum_out  (ACT, natural layout)
    ctT = XBAR DMA transpose of c~ (padded to 2432)
  s-pass: x~T = uT (.) ctT (one DVE mul, i-broadcast by stride-0 AP),
  then 76 accumulating matmuls in transposed [j,b] layout (no per-
  iteration PE transposes); squash in natural b-partition space.

All matmul contraction operands sit at partition base 0 (nonzero PE
row-base crashes the device). r padded 2336->2432 (19 groups of 128).
uT chunks keyed (i, rg), partition rp = r - 128*rg, built by XBAR DMA
transposes from i-planar bf16 u. wotN = W2^T [32=(k,o), (i,r)] built by
76 PE transposes of w2p; PM matmuls contract 32 rows against MpadT_k
whose other-class half is zero.
"""

import numpy as np

import bass_rust
import concourse.bass as bass
import concourse.mybir as mybir
from concourse import tile
from concourse.bass_utils import run_bass_kernel_spmd

# problem dims (hardcoded per contest rules)
B, R, Cin, K, Cout = 1024, 2336, 4, 2, 16
NCORES = 8
BL = B // NCORES          # 128 batch rows per core
RG = 19                   # r-groups of 128 (last holds 32 valid rows)
RPAD = RG * 128           # 2432
J2 = Cin * RPAD           # 9728 padded contraction length
KO = K * Cout             # 32
RFULL = 128 * (RG - 1)    # 2304
# per-(group, class) ws-mul routing: D=DVE direct, A=ACT evac + 2x DVE mul,
# P=ACT evac + Pool mul (cycled over groups)
POLICY = ("DA", "AD")

F32 = mybir.dt.float32
BF16 = mybir.dt.bfloat16
AF = mybir.ActivationFunctionType
OP = mybir.AluOpType


def _split_ctrl_waits(nc, max_waits=1):
    """walrus rejects >1 sync-wait per instruction; hoist extras onto
    single-wait NoOps inserted just before (same engine, program order)."""
    for fn in nc.m.functions:
        for bb in fn.blocks:
            out, changed = [], False
            for ins in bb.instructions:
                si = ins.sync_info
                if (
                    si is not None
                    and si.on_wait is not None
                    and len(si.on_wait) > max_waits
                ):
                    waits = list(si.on_wait)
                    for j, w in enumerate(waits[:-1]):
                        out.append(
                            mybir.InstNoOp(
                                name=f"{ins.name}-waitsplit-{j}",
                                engine=ins.engine,
                                ins=[],
                                outs=[],
                                sync_info=bass_rust.SyncInfo(on_wait=[w], on_update=[]),
                            )
                        )
                    ins.sync_info = bass_rust.SyncInfo(
                        on_wait=[waits[-1]], on_update=list(si.on_update or [])
                    )
                    changed = True
                out.append(ins)
            if changed:
                bb.instructions = out


def build_nc(debug=(), nrep=1):
    nc = bass.Bass()
    u_d = nc.declare_dram_parameter("u", [BL, R, Cin], F32, isOutput=False)
    w_d = nc.declare_dram_parameter("W", [K, R, Cin, Cout], F32, isOutput=False)
    out_d = nc.declare_dram_parameter("out", [BL, K], F32, isOutput=True)
    dbg_d = {
        name: nc.declare_dram_parameter(name, shape, F32, isOutput=True)
        for name, shape in debug
    }

    with tile.TileContext(nc) as tc:
        with (
            tc.tile_pool(name="big", bufs=1) as big,
            tc.tile_pool(name="small", bufs=1) as small,
            tc.tile_pool(name="pm", bufs=2, space=bass.MemorySpace.PSUM) as pmp,
            tc.tile_pool(name="wps", bufs=2, space=bass.MemorySpace.PSUM) as wpsp,
            tc.tile_pool(name="psm", bufs=1, space=bass.MemorySpace.PSUM) as psm,
        ):
            # ---------- persistent SBUF tiles ----------
            scr = big.tile([128, J2], F32, tag="scr")       # u f32 stage -> ws0/ws1
            u_im = big.tile([128, Cin, RPAD], BF16, tag="u_im")   # i-planar
            uT = big.tile([128, Cin, RG, 128], BF16, tag="uT")
            w2pf = big.tile([128, RG, Cin, KO], F32, tag="w2pf")
            w2p = big.tile([128, RG, Cin, KO], BF16, tag="w2p")
            wotN = big.tile([32, Cin, RG, 128], BF16, tag="wotN")
            xt = [big.tile([128, Cin, RG, 128], BF16, name=f"xt{k}", tag=f"xt{k}")
                  for k in range(K)]
            ctN = [big.tile([128, RPAD], BF16, name=f"ctN{k}", tag=f"ctN{k}")
                   for k in range(K)]
            ctT = [big.tile([128, RG, 128], BF16, name=f"ctT{k}", tag=f"ctT{k}")
                   for k in range(K)]
            dt = [[big.tile([128, RPAD], BF16, name=f"dt{k}{h}", tag=f"dt{k}{h}")
                   for h in range(2)] for k in range(K)]
            MpadT = [big.tile([32, 128], BF16, name=f"MpadT{k}", tag=f"MpadT{k}")
                     for k in range(K)]
            pm_sb = [big.tile([128, 1024], BF16, name=f"pmsb{j}", tag=f"pmsb{j}")
                     for j in range(2)]
            snap = big.tile([128, RG, 128], BF16, tag="snap")
            snap2 = big.tile([128, RPAD], BF16, tag="snap2")

            iota32 = small.tile([128, 128], mybir.dt.int32, tag="iota")
            id_bf = small.tile([128, 128], BF16, tag="id_bf")
            id_f32 = small.tile([128, 128], F32, tag="id_f32")
            Mpair = [small.tile([128, KO], BF16, name=f"Mpair{k}", tag=f"Mpair{k}")
                     for k in range(K)]
            s_sb1 = small.tile([32, 128], F32, tag="s_sb1")
            s_sbk = [small.tile([16, 128], F32, name=f"s_sbk{k}", tag=f"s_sbk{k}")
                     for k in range(K)]
            sqj = small.tile([128, 16], F32, tag="sqj")
            nraw = small.tile([128, 8], F32, tag="nraw")
            onepn = small.tile([128, 8], F32, tag="onepn")
            ripn = small.tile([128, 8], F32, tag="ripn")
            tau = small.tile([128, 8], F32, tag="tau")
            gz = small.tile([128, 8], F32, tag="gz")
            zab = small.tile([128, 8], F32, tag="zab")
            rZ = small.tile([128, 4], F32, tag="rZ")
            cls = small.tile([128, K], F32, tag="cls")
            clse = small.tile([128, K], F32, tag="clse")
            clsum = small.tile([128, 1], F32, tag="clsum")
            rcs = small.tile([128, 1], F32, tag="rcs")
            outt = small.tile([128, K], F32, tag="outt")

            # ws views overlaying scr (f32 tile viewed as 2x bf16 regions)
            scr_bf = scr[:].bitcast(BF16)          # [128, 2*J2]
            wsf = [scr_bf[:, k * J2:(k + 1) * J2] for k in range(K)]
            u_imf = u_im[:].rearrange("b i r -> b (i r)")

            chunks = [(i, rg) for rg in range(RG) for i in range(Cin)]
            HG = ((0, 10), (10, RG))               # rg halves
            HR = ((0, 1280), (1280, R))            # r halves (valid region)

            def kpart(rg):
                return 32 if rg == RG - 1 else 128

            def emit_body(rep):
                # ---------- identities / constants / pads ----------
                nc.gpsimd.iota(
                    iota32[:], pattern=[[1, 128]], base=0, channel_multiplier=-1
                )
                nc.vector.tensor_scalar(id_bf[:], iota32[:], 0, None, op0=OP.is_equal)
                nc.vector.tensor_scalar(id_f32[:], iota32[:], 0, None, op0=OP.is_equal)
                for i in range(Cin):
                    nc.gpsimd.memset(u_im[:, i, R:], 0.0)
                nc.gpsimd.memset(w2pf[:], 0.0)
                for k in range(K):
                    # zero the other class's half; M-updates only touch own half
                    nc.gpsimd.memset(Mpair[k][:, 16 * (1 - k):16 * (2 - k)], 0.0)
                    nc.gpsimd.memset(ctN[k][:, R:], 0.0)

                # ---------- loads: u halves lead, W fills the gaps ----------
                uflat = u_d[:].rearrange("b r i -> b (r i)")
                u_f = scr[:, :R * Cin]
                HCOLS = ((0, 5120), (5120, R * Cin))
                for h in range(2):
                    c0, c1 = HCOLS[h]
                    mid = (c0 + c1) // 2 // 4 * 4
                    nc.sync.dma_start(out=u_f[:, c0:mid], in_=uflat[:, c0:mid])
                    nc.scalar.dma_start(out=u_f[:, mid:c1], in_=uflat[:, mid:c1])
                    if h == 0:
                        # raw load w2pf[rp, (k, rg, i, o)] (2+2 DMAs); the
                        # bf16 cast permutes to [rp, rg, i, 16k+o]
                        w2pf_flat = w2pf[:].rearrange(
                            "rp rg i ko -> rp (rg i ko)")
                        wraw = w2pf_flat.rearrange(
                            "rp (k rg io) -> rp k rg io", k=K, rg=RG
                        )
                        for k in range(K):
                            nc.scalar.dma_start(
                                out=wraw[:, k, :RG - 1],
                                in_=w_d[k, :RFULL].rearrange(
                                    "(rg rp) i o -> rp rg (i o)", rp=128
                                ),
                            )
                            nc.scalar.dma_start(
                                out=wraw[:32, k, RG - 1],
                                in_=w_d[k, RFULL:].rearrange(
                                    "rp i o -> rp (i o)"),
                            )

                uTf = uT[:].rearrange("b i rg q -> b (i rg q)")
                for h in range(2):
                    r0, r1 = HR[h]
                    rm = (r0 + r1) // 2
                    # cast to i-planar bf16: u_im[:, i, r] = u_f[:, 4r+i]
                    nc.vector.tensor_copy(
                        u_im[:, :, r0:rm],
                        u_f[:, 4 * r0:4 * rm].rearrange("b (r i) -> b i r", i=Cin),
                    )
                    nc.scalar.copy(
                        out=u_im[:, :, rm:r1],
                        in_=u_f[:, 4 * rm:4 * r1].rearrange(
                            "b (r i) -> b i r", i=Cin),
                    )
                    g0, g1 = HG[h]
                    # uT chunks via PE transposes + split evacs (PE is idle
                    # during the load; device-verified path)
                    usel = [(i, rg) for i in range(Cin) for rg in range(g0, g1)]
                    for e0 in range(0, len(usel), 8):
                        en = min(8, len(usel) - e0)
                        ups = wpsp.tile([128, 1024], BF16, tag="upt",
                                        bufs=2)
                        for j in range(en):
                            i, rg = usel[e0 + j]
                            nc.tensor.transpose(
                                ups[:, 128 * j:128 * (j + 1)],
                                u_im[:, i, 128 * rg:128 * (rg + 1)],
                                id_bf[:],
                            )
                        j = 0
                        while j < en:
                            i0 = usel[e0 + j][0]
                            j2 = j
                            while j2 + 1 < en and usel[e0 + j2 + 1][0] == i0:
                                j2 += 1
                            c0 = usel[e0 + j][0] * RG + usel[e0 + j][1]
                            if (e0 // 8) % 2 == 0:
                                nc.vector.tensor_copy(
                                    uTf[:, 128 * c0:128 * (c0 + j2 - j + 1)],
                                    ups[:, 128 * j:128 * (j2 + 1)],
                                )
                            else:
                                nc.scalar.copy(
                                    out=uTf[:, 128 * c0:128 * (c0 + j2 - j + 1)],
                                    in_=ups[:, 128 * j:128 * (j2 + 1)],
                                )
                            j = j2 + 1
                    if h == 0:
                        nc.vector.tensor_copy(
                            w2p[:].rearrange(
                                "rp rg i (k o) -> rp rg i k o", k=K),
                            w2pf_flat.rearrange(
                                "rp (k rg i o) -> rp rg i k o",
                                k=K, rg=RG, i=Cin
                            ),
                        )
                        # wotN[(16k+o), i, rg, rp] = W2^T via PE transposes
                        gsz = 8
                        for g0w in range(0, len(chunks), gsz):
                            gn = min(gsz, len(chunks) - g0w)
                            wps = wpsp.tile([32, 1024], BF16, tag="wps",
                                            bufs=1)
                            for j in range(gn):
                                i, rg = chunks[g0w + j]
                                nc.tensor.transpose(
                                    wps[:, 128 * j:128 * (j + 1)],
                                    w2p[:, rg, i, :],
                                    id_bf[:],
                                )
                            rg0w = chunks[g0w][1]
                            nrgw = gn // Cin
                            dst = wotN[:, :, rg0w:rg0w + nrgw, :].rearrange(
                                "p i g q -> p g i q"
                            )
                            nc.scalar.copy(
                                out=dst,
                                in_=wps[:, :128 * gn].rearrange(
                                    "p (g i q) -> p g i q", g=nrgw, i=Cin),
                            )

                def s_pass(acc, moving, h, ko_sl=slice(0, KO)):
                    g0, g1 = HG[h]
                    sel = [(i, rg) for i in range(Cin) for rg in range(g0, g1)]
                    for idx, (i, rg) in enumerate(sel):
                        kp = kpart(rg)
                        nc.tensor.matmul(
                            acc,
                            w2p[:kp, rg, i, ko_sl],
                            moving[:kp, i, rg, :],
                            start=(h == 0 and idx == 0),
                            stop=(h == 1 and idx == len(sel) - 1),
                        )

                def squash(t, k, tp_ap, zs):
                    """tp_ap: PSUM [128, 16] f32 = s~^T cols for class k.
                    zs: float or AP [128,1] = 1/Z."""
                    c = slice(2 * (t - 1) + k, 2 * (t - 1) + k + 1)
                    nc.scalar.activation(
                        sqj[:], tp_ap, AF.Square, scale=zs, accum_out=nraw[:, c]
                    )
                    nc.vector.tensor_scalar_add(onepn[:, c], nraw[:, c], 1.0)
                    nc.vector.reciprocal(ripn[:, c], onepn[:, c])
                    if t < 3:
                        nc.scalar.activation(tau[:, c], nraw[:, c], AF.Sqrt)
                        nc.vector.tensor_mul(gz[:, c], tau[:, c], ripn[:, c])
                        if isinstance(zs, float):
                            nc.vector.tensor_scalar_mul(gz[:, c], gz[:, c], zs)
                        else:
                            nc.vector.tensor_mul(gz[:, c], gz[:, c], zs)
                        if t == 1:
                            nc.vector.tensor_scalar_mul(
                                Mpair[k][:, 16 * k:16 * (k + 1)], tp_ap, gz[:, c]
                            )
                        else:
                            nc.vector.scalar_tensor_tensor(
                                out=Mpair[k][:, 16 * k:16 * (k + 1)],
                                in0=tp_ap,
                                scalar=gz[:, c],
                                in1=Mpair[k][:, 16 * k:16 * (k + 1)],
                                op0=OP.mult,
                                op1=OP.add,
                            )
                        # MpadT[k] = Mpair[k]^T (other half zero)
                        mt = wpsp.tile([32, 1024], BF16, tag="wps", bufs=1)
                        nc.tensor.transpose(mt[:, :128], Mpair[k][:], id_bf[:])
                        nc.vector.tensor_copy(MpadT[k][:], mt[:, :128])
                    else:
                        nc.vector.tensor_mul(cls[:, k:k + 1], nraw[:, c], ripn[:, c])

                wsv = [wsf[k].rearrange("b (i rg q) -> b i rg q", i=Cin, rg=RG)
                       for k in range(K)]

                def pm_groups(k, glist, policy):
                    """PM outer + ws mul for class k over rg-major groups.
                    Each group = rg-pair x 4 i (or the rg=18 quad)."""
                    for gi, (c0, cn) in enumerate(glist):
                        pmt = pmp.tile([128, 1024], F32, tag="pmt")
                        for j in range(cn):
                            i, rg = chunks[c0 + j]
                            nc.tensor.matmul(
                                pmt[:, 128 * j:128 * (j + 1)],
                                MpadT[k][:],
                                wotN[:, i, rg, :],
                                start=True,
                                stop=True,
                            )
                        rg0 = chunks[c0][1]
                        nrg = cn // Cin
                        uts = u_im[:, :, 128 * rg0:128 * (rg0 + nrg)].rearrange(
                            "b i (g q) -> b g i q", g=nrg
                        )
                        wss = wsv[k][:, :, rg0:rg0 + nrg].rearrange(
                            "b i g q -> b g i q"
                        )
                        pol = policy[gi % len(policy)][k]
                        if pol == "D":
                            nc.vector.tensor_mul(
                                wss, uts,
                                pmt[:, :128 * cn].rearrange(
                                    "b (g i q) -> b g i q", g=nrg, i=Cin),
                            )
                        else:
                            sb = pm_sb[(2 * gi + k) % 2]
                            nc.scalar.copy(out=sb[:, :128 * cn],
                                           in_=pmt[:, :128 * cn])
                            eng = nc.vector if pol == "A" else nc.gpsimd
                            eng.tensor_mul(
                                wss, uts,
                                sb[:, :128 * cn].rearrange(
                                    "b (g i q) -> b g i q", g=nrg, i=Cin),
                            )

                def l_chain(t, k, h):
                    """segreduce half -> exp half (ACT) -> XBAR (DMA)."""
                    r0, r1 = HR[h]
                    d0, d1 = dt[k]
                    nc.vector.tensor_add(
                        d0[:, r0:r1], wsf[k][:, r0:r1],
                        wsf[k][:, RPAD + r0:RPAD + r1])
                    nc.vector.tensor_add(
                        d1[:, r0:r1], wsf[k][:, 2 * RPAD + r0:2 * RPAD + r1],
                        wsf[k][:, 3 * RPAD + r0:3 * RPAD + r1])
                    nc.vector.tensor_add(d0[:, r0:r1], d0[:, r0:r1],
                                         d1[:, r0:r1])
                    nc.scalar.activation(
                        ctN[k][:, r0:r1], d0[:, r0:r1], AF.Exp,
                        accum_out=zab[:, 2 * k + h:2 * k + h + 1],
                    )
                    g0, g1 = HG[h]
                    eng = nc.sync if k == 0 else nc.scalar
                    eng.dma_start_transpose(
                        ctT[k][:, g0:g1, :], ctN[k][:, 128 * g0:128 * g1]
                    )

                def pm_phase(t, policy):
                    """Both classes' PM field, halves pipelined: the h0
                    L-chain overlaps the h1 PM groups."""
                    GL_H0 = [(8 * g, 8) for g in range(5)]           # rg 0..9
                    GL_H1 = [(40 + 8 * g, 8) for g in range(4)] + [(72, 4)]
                    for k in range(K):
                        pm_groups(k, GL_H0, policy)
                    for k in range(K):
                        pm_groups(k, GL_H1, policy)
                    for k in range(K):
                        l_chain(t, k, 0)
                    for k in range(K):
                        l_chain(t, k, 1)
                    for k in range(K):
                        zc = slice(2 * (t - 1) + k, 2 * (t - 1) + k + 1)
                        nc.vector.tensor_add(zab[:, 2 * k:2 * k + 1],
                                             zab[:, 2 * k:2 * k + 1],
                                             zab[:, 2 * k + 1:2 * k + 2])
                        nc.vector.reciprocal(rZ[:, zc], zab[:, 2 * k:2 * k + 1])

                def modulate(k, h):
                    g0, g1 = HG[h]
                    nc.vector.tensor_mul(
                        xt[k][:, :, g0:g1, :],
                        uT[:, :, g0:g1, :],
                        ctT[k][:, g0:g1, :].unsqueeze(1).broadcast_to(
                            [128, Cin, g1 - g0, 128]
                        ),
                    )

                def sq_evac(k, acc_ap, zt):
                    """acc_ap [16,128] psum -> tp [128,16] psum via SBUF.
                    tp slots live in separate 512B PSUM zero-regions: a
                    start=True write appears to clear its whole region."""
                    nc.scalar.copy(out=s_sbk[k][:], in_=acc_ap)
                    tp = zt[:, 384 + 16 * k:384 + 16 * (k + 1)]
                    nc.tensor.transpose(tp, s_sbk[k][:], id_f32[:16, :16])
                    return tp

                # zt bank map: acc1 [0:128] (32 rows), accs k0 [128:256],
                # k1 [256:384] (16 rows, reused t2/t3), tp k0 [384:400],
                # tp k1 [400:416], tp2 [416:448]
                zt = psm.tile([128, 512], F32, tag="zt", bufs=1)
                accs = [zt[:16, 128:256], zt[:16, 256:384]]

                # ================= t = 1 =================
                acc1 = zt[:32, 0:128]
                for h in range(2):
                    s_pass(acc1, uT, h)
                nc.scalar.copy(out=s_sb1[:], in_=acc1)
                tp2 = zt[:, 416:448]
                nc.tensor.transpose(tp2, s_sb1[:], id_f32[:32, :32])
                GL_ALL = ([(8 * g, 8) for g in range(9)] + [(72, 4)])
                for k in range(K):
                    squash(1, k, tp2[:, 16 * k:16 * (k + 1)], 1.0 / R)
                pm_phase(1, POLICY)
                if debug:
                    nc.vector.tensor_copy(snap[:], ctT[0][:])
                    nc.vector.tensor_copy(snap2[:], ctN[0][:])

                # ================= t = 2 =================
                for k in range(K):
                    modulate(k, 0)
                for k in range(K):
                    for h in range(2):
                        if h == 1:
                            modulate(k, 1)
                        s_pass(accs[k], xt[k], h,
                               ko_sl=slice(16 * k, 16 * (k + 1)))
                for k in range(K):
                    tp = sq_evac(k, accs[k], zt)
                    squash(2, k, tp, rZ[:, k:k + 1])
                pm_phase(2, POLICY)

                # ================= t = 3 =================
                for k in range(K):
                    modulate(k, 0)
                for k in range(K):
                    for h in range(2):
                        if h == 1:
                            modulate(k, 1)
                        s_pass(accs[k], xt[k], h,
                               ko_sl=slice(16 * k, 16 * (k + 1)))
                for k in range(K):
                    tp = sq_evac(k, accs[k], zt)
                    squash(3, k, tp, rZ[:, 2 + k:3 + k])

                # out = softmax over k of classes
                nc.scalar.activation(clse[:], cls[:], AF.Exp)
                nc.vector.tensor_add(clsum[:], clse[:, 0:1], clse[:, 1:2])
                nc.vector.reciprocal(rcs[:], clsum[:])
                nc.vector.tensor_scalar_mul(outt[:], clse[:], rcs[:])
                nc.sync.dma_start(out=out_d[:], in_=outt[:])

                for name, _ in debug:
                    srcs = {
                        "dbg_L0": dt[0][0], "dbg_L1": dt[1][0], "dbg_cls": cls,
                        "dbg_M0": Mpair[0], "dbg_M1": Mpair[1], "dbg_rZ": rZ,
                        "dbg_ct0": ctN[0], "dbg_ct1": ctN[1], "dbg_n": nraw, "dbg_gz": gz, "dbg_ctT0": ctT[0], "dbg_xt0": xt[0], "dbg_snap": snap, "dbg_snap2": snap2,
                    }[name]
                    ap = srcs[:]
                    if ap.dtype == BF16:
                        ap = ap.bitcast(F32)
                    nc.sync.dma_start(out=dbg_d[name][:], in_=ap)

            for _rep in range(nrep):
                emit_body(_rep)

    _split_ctrl_waits(nc)
    return nc


_CACHED = {}


def _get_nc(debug=(), nrep=1):
    key = (tuple(debug), nrep)
    if key not in _CACHED:
        _CACHED[key] = build_nc(debug, nrep=nrep)
    return _CACHED[key]


def kernel(u: np.ndarray, W: np.ndarray, debug=(), trace=False):
    u = np.ascontiguousarray(u, dtype=np.float32)
    W = np.ascontiguousarray(W, dtype=np.float32)
    assert u.shape == (B, R, Cin) and W.shape == (K, R, Cin, Cout)
    nc = _get_nc(debug)
    in_maps = [
        {"u": u[i * BL:(i + 1) * BL], "W": W} for i in range(NCORES)
    ]
    res = run_bass_kernel_spmd(nc, in_maps, core_ids=list(range(NCORES)), trace=trace)
    out = np.concatenate([res.results[i]["out"] for i in range(NCORES)], axis=0)
    if debug or trace:
        return out, res
    return out
